# revision 25
# baseline (speedup 1.0000x reference)
"""GQA causal attention (B=2, S=2048, D=2048, 16 q heads / 4 kv heads, RoPE)
for 8 Trainium2 NeuronCores.

Sharding: core i = (batch b = i//4, kv-head group g = i%4). Each core computes
its group's Q/K/V projections, RoPE, causal attention and the partial output
projection; the host sums the 4 per-group partials per batch.

On-core layout is fully "transposed" (features on partitions):
  xT [D, S], QT/KT [d, S] -> QK scores land as [k, q], softmax runs along k
  (partitions) with the denominator computed by an all-ones matmul, and PV
  accumulates out^T [d, q] directly in PSUM. The final projection contracts
  over the group's 512 head-dims on partitions.
All data is bf16 (fp32r runs in single-pass HIGH mode anyway, so precision is
comparable); matmuls accumulate in fp32 PSUM. bf16 halves HBM traffic and has
no moving>=256 constraint, so causal band tiles are trimmed to exact width.

The causal mask is folded into the score PSUM by an identity-stationary
matmul that accumulates -1e5 at masked positions, so exp output is already
masked (zero there) and the PV matmul never waits on a vector-engine mask.
Softmax denominators batch on the DVE in bf16: the 4 band tiles sum into one
accumulator (one ones-matmul), full tiles sum in pairs.
q/k/v live in per-slice tiles so slice-0 attention never waits on slice-3
RoPE or transposes.
"""

import sys
import types

sys.path.insert(0, "/opt/trn_rl_repo")

# If tracing is ever requested (e.g. BASS_TRACE=1 in the environment),
# concourse needs antenv.axon_hooks, which this image lacks; provide it.
try:
    import antenv  # noqa: F401

    if "antenv.axon_hooks" not in sys.modules:
        from trn_agent_boot.trn_boot import _ntff_profile_via_ctypes

        _mod = types.ModuleType("antenv.axon_hooks")
        _hook = _ntff_profile_via_ctypes("/opt/axon/libaxon_pjrt.so")
        _mod.get_axon_ntff_profile_hook = lambda: _hook
        sys.modules["antenv.axon_hooks"] = _mod
except Exception:
    pass

import ml_dtypes
import numpy as np
from contextlib import ExitStack

import concourse.bacc as bacc
import concourse.mybir as mybir
import concourse.tile as tile
from concourse.bass_utils import run_bass_kernel_spmd

B, S, DIM = 2, 2048, 2048
N_HEADS, N_KV, HD = 16, 4, 128
HPG = N_HEADS // N_KV      # q heads per kv group
GD = HPG * HD              # 512 = group width
P = 128
NS = S // 512              # 4 s-slices of 512
NC = DIM // P              # 16 contraction chunks of 128
NKT = S // P               # 16 k tiles
F32 = mybir.dt.float32
BF16 = mybir.dt.bfloat16
NPBF16 = ml_dtypes.bfloat16
SCALE = 1.0 / float(np.sqrt(HD))
MASK_NEG = -100000.0

# bf16 consts column layout
C_RT = 0          # [128]  RoPE rotation (R.T)
C_ID = 128        # [128]  identity
C_ONES = 256      # [128]  all-ones
C_COS = 384       # [2048] cos, repeated x2 along d
C_SIN = 2432      # [2048]
C_AM = 4480       # [4*512] causal band masks, additive 0/-1e5
NCONST = 6528

_CACHE = {}


def _build():
    nc = bacc.Bacc()
    xT = nc.dram_tensor("xT", [DIM, S], BF16, kind="ExternalInput")
    wqT = nc.dram_tensor("wqT", [DIM, GD], BF16, kind="ExternalInput")
    wkT = nc.dram_tensor("wkT", [DIM, HD], BF16, kind="ExternalInput")
    wvT = nc.dram_tensor("wvT", [DIM, HD], BF16, kind="ExternalInput")
    woT = nc.dram_tensor("woT", [GD, DIM], BF16, kind="ExternalInput")
    consts = nc.dram_tensor("consts", [P, NCONST], BF16, kind="ExternalInput")
    out = nc.dram_tensor("out", [S, DIM], BF16, kind="ExternalOutput")

    EXP = mybir.ActivationFunctionType.Exp

    with tile.TileContext(nc) as tc, ExitStack() as ctx:
        cpool = ctx.enter_context(tc.tile_pool(name="consts", bufs=1))
        persist = ctx.enter_context(tc.tile_pool(name="persist", bufs=1))

        consts_sb = cpool.tile([P, NCONST], BF16, name="consts_sb")
        rt = consts_sb[:, C_RT:C_RT + 128]
        ident = consts_sb[:, C_ID:C_ID + 128]
        ones_bf = consts_sb[:, C_ONES:C_ONES + 128]
        cosf = consts_sb[:, C_COS:C_COS + S]
        sinf = consts_sb[:, C_SIN:C_SIN + S]
        amasks = consts_sb[:, C_AM:C_AM + 4 * 512].rearrange("p (r q) -> p r q", r=4)

        wo_sb = persist.tile([P, HPG, DIM], BF16, name="wo_sb")
        # per-slice q/k/v tiles: slice j's attention only depends on slice j's
        # RoPE/transposes, not on the last phase-1 write to one big tile
        qs = [persist.tile([P, HPG, 512], BF16, name=f"qs{j}") for j in range(NS)]
        ks = [persist.tile([P, 512], BF16, name=f"ks{j}") for j in range(NS)]
        vs = [persist.tile([P, 4, HD], BF16, name=f"vs{j}") for j in range(NS)]
        vt_sb = persist.tile([P, S], BF16, name="vt_sb")

        def ktile(kt):
            return ks[kt // 4][:, P * (kt % 4):P * (kt % 4 + 1)]

        def vtile(kt):
            return vs[kt // 4][:, kt % 4, :]

        # ---- Phase 1: QKV projections + RoPE + V transpose, per s-slice ----
        with ExitStack() as p1:
            wpool = p1.enter_context(tc.tile_pool(name="wqkv", bufs=1))
            xpool = p1.enter_context(tc.tile_pool(name="xs", bufs=3))
            tmpp = p1.enter_context(tc.tile_pool(name="ropetmp", bufs=3))
            psA = p1.enter_context(tc.tile_pool(name="psA", bufs=1, space="PSUM"))
            psRT = p1.enter_context(tc.tile_pool(name="psRT", bufs=1, space="PSUM"))

            wq_sb = wpool.tile([P, NC, GD], BF16, name="wq_sb")
            wk_sb = wpool.tile([P, NC, HD], BF16, name="wk_sb")
            wv_sb = wpool.tile([P, NC, HD], BF16, name="wv_sb")

            warm_sb = wpool.tile([P, 512], BF16, name="warm_sb")
            nc.vector.memset(warm_sb, 0.0)
            for _ in range(3):
                wps = psRT.tile([P, 512], F32, name="warm_ps", tag="rt")
                nc.tensor.matmul(wps, warm_sb[:, :P], warm_sb, start=True, stop=True)

            def dma_consts(lo, n):
                nc.sync.dma_start(out=consts_sb[:, lo:lo + n],
                                  in_=consts[:, lo:lo + n])

            def dma_wq_chunk(cc):
                nc.sync.dma_start(
                    out=wq_sb[:, 4 * cc:4 * (cc + 1), :],
                    in_=wqT[512 * cc:512 * (cc + 1), :]
                    .rearrange("(c p) h -> p c h", p=P))

            def dma_xs(xs, j, cc, engine=None):
                (engine or nc.sync).dma_start(
                    out=xs,
                    in_=xT[512 * cc:512 * (cc + 1), 512 * j:512 * (j + 1)]
                    .rearrange("(c p) s -> p c s", p=P))

            prefetched = None
            for j in range(NS):
                ps = [psA.tile([P, 512], F32, name=f"proj{t}",
                               bufs=2 if t == 0 else 1) for t in range(6)]
                j0_tiles = {}
                for cc in range(4):
                    if cc == 0 and j > 0:
                        xs = prefetched
                    elif j == 0 and cc in j0_tiles:
                        xs = j0_tiles[cc]
                    else:
                        xs = xpool.tile([P, 4, 512], BF16, name="xs")
                    if j == 0 and cc == 0:
                        # interleave the first x columns and q-weight chunks in
                        # small pieces, issued from three engines in parallel
                        # (SP descriptor generation is ~1us per DMA)
                        for c4 in range(4):
                            nc.sync.dma_start(
                                out=xs[:, c4, :],
                                in_=xT[128 * c4:128 * (c4 + 1), 0:512]
                                .rearrange("(c p) s -> p c s", p=P)[:, 0, :])
                            nc.gpsimd.dma_start(
                                out=wq_sb[:, c4, :],
                                in_=wqT[128 * c4:128 * (c4 + 1), :]
                                .rearrange("(c p) h -> p c h", p=P)[:, 0, :])
                        nc.scalar.dma_start(
                            out=wk_sb, in_=wkT[:, :]
                            .rearrange("(c p) h -> p c h", p=P))
                        nc.scalar.dma_start(
                            out=wv_sb, in_=wvT[:, :]
                            .rearrange("(c p) h -> p c h", p=P))
                        nc.gpsimd.dma_start(
                            out=consts_sb[:, C_RT:C_RT + 384],
                            in_=consts[:, C_RT:C_RT + 384])
                        nc.scalar.dma_start(
                            out=consts_sb[:, C_COS:C_COS + 512],
                            in_=consts[:, C_COS:C_COS + 512])
                        nc.gpsimd.dma_start(
                            out=consts_sb[:, C_SIN:C_SIN + 512],
                            in_=consts[:, C_SIN:C_SIN + 512])
                    elif not (j == 0 and cc in j0_tiles) and not (cc == 0 and j > 0):
                        dma_xs(xs, j, cc)
                    if j == 0 and cc + 1 < 4 and cc + 1 not in j0_tiles:
                        nxt = xpool.tile([P, 4, 512], BF16, name="xs")
                        dma_xs(nxt, 0, cc + 1,
                               engine=(None, nc.scalar, nc.gpsimd)[cc])
                        dma_wq_chunk(cc + 1)
                        j0_tiles[cc + 1] = nxt
                    for c4 in range(4):
                        c = 4 * cc + c4
                        first = c == 0
                        last = c == NC - 1
                        for t in range(HPG):
                            nc.tensor.matmul(
                                ps[t], wq_sb[:, c, 128 * t:128 * (t + 1)],
                                xs[:, c4, :], start=first, stop=last)
                        nc.tensor.matmul(ps[4], wk_sb[:, c, :], xs[:, c4, :],
                                         start=first, stop=last)
                        nc.tensor.matmul(ps[5], wv_sb[:, c, :], xs[:, c4, :],
                                         start=first, stop=last)
                if j + 1 < NS:
                    xs_pre = xpool.tile([P, 4, 512], BF16, name="xs")
                    dma_xs(xs_pre, j + 1, 0)
                    prefetched = xs_pre
                else:
                    prefetched = None
                for t in range(HPG):
                    nc.scalar.copy(qs[j][:, t, :], ps[t])
                nc.scalar.copy(ks[j], ps[4])
                nc.scalar.copy(vt_sb[:, 512 * j:512 * (j + 1)], ps[5])
                if j == 0:
                    # attention constants + next slice's cos/sin
                    dma_consts(C_AM, 4 * 512)
                elif j == 1:
                    nc.sync.dma_start(
                        out=wo_sb,
                        in_=woT[:, :].rearrange("(c p) e -> p c e", p=P))
                if j + 1 < NS:
                    dma_consts(C_COS + 512 * (j + 1), 512)
                    dma_consts(C_SIN + 512 * (j + 1), 512)

                # RoPE for this slice (4 q heads + k); the cos-mul runs on
                # gpsimd so the DVE only carries the rot*sin mul and the add
                sl = slice(512 * j, 512 * (j + 1))
                for t in range(HPG + 1):
                    src = qs[j][:, t, :] if t < HPG else ks[j]
                    t2 = tmpp.tile([P, 512], BF16, name="t2")
                    nc.gpsimd.tensor_mul(t2, src, cosf[:, sl])
                    rot = psRT.tile([P, 512], F32, name="rot", tag="rt")
                    nc.tensor.matmul(rot, rt, src, start=True, stop=True)
                    t1 = tmpp.tile([P, 512], F32, name="t1")
                    nc.vector.tensor_mul(t1, rot, sinf[:, sl])
                    nc.vector.tensor_add(src, t1, t2)

                # V transpose (slice 0 only; kt 4..15 run as phase-2
                # filler work during the latency-bound j=0/j=1 heads)
                if j == 0:
                    for kt in range(4):
                        trb = psRT.tile([P, 512], F32, name="trb", tag="rt")
                        tr = trb.bitcast(BF16)[:, :P]
                        nc.tensor.transpose(tr, vt_sb[:, P * kt:P * (kt + 1)],
                                            ident)
                        nc.scalar.copy(vtile(kt), tr)

        # ---- Phase 2: attention (j outer) + interleaved output projection ----
        with ExitStack() as p3:
            ppool = p3.enter_context(tc.tile_pool(name="ptiles", bufs=12))
            bcpool = p3.enter_context(tc.tile_pool(name="bc", bufs=4))
            attnp = p3.enter_context(tc.tile_pool(name="attn", bufs=1))
            outp = p3.enter_context(tc.tile_pool(name="outp", bufs=6))
            psQK = p3.enter_context(tc.tile_pool(name="psQK", bufs=4, space="PSUM"))
            psPV = p3.enter_context(tc.tile_pool(name="psPV", bufs=1, space="PSUM"))
            psDN = p3.enter_context(tc.tile_pool(name="psDN", bufs=1, space="PSUM"))
            psO = p3.enter_context(tc.tile_pool(name="psO", bufs=2, space="PSUM"))

            attn_sb = attnp.tile([P, HPG, S], BF16, name="attn_sb")

            fillers = []

            def make_unit(st, e):
                def unit():
                    ops = psO.tile([P, 512], F32, name="ops")
                    for hc in range(HPG):
                        nc.tensor.matmul(
                            ops, attn_sb[:, hc, P * st:P * (st + 1)],
                            wo_sb[:, hc, 512 * e:512 * (e + 1)],
                            start=(hc == 0), stop=(hc == HPG - 1))
                    osb = outp.tile([P, 512], BF16, name="osb")
                    nc.vector.tensor_copy(osb, ops)
                    eng = nc.sync if (st + e) % 2 == 0 else nc.gpsimd
                    eng.dma_start(
                        out=out[P * st:P * (st + 1), 512 * e:512 * (e + 1)],
                        in_=osb)
                return unit

            tfillers = list(range(4, NKT))

            def emit_transpose():
                kt = tfillers.pop(0)
                trb = psO.tile([P, 512], F32, name="ops")
                tr = trb.bitcast(BF16)[:, :P]
                nc.tensor.transpose(tr, vt_sb[:, P * kt:P * (kt + 1)], ident)
                nc.vector.tensor_copy(vtile(kt), tr)

            def emit_filler():
                if tfillers:
                    emit_transpose()
                    if tfillers:
                        emit_transpose()
                elif fillers:
                    fillers.pop(0)()

            for j in range(NS):
                sl = slice(512 * j, 512 * (j + 1))
                nkt = 4 * (j + 1)
                for h in range(HPG):
                    pv = psPV.tile([P, 512], F32, name="pv")
                    den = psDN.tile([P, 512], F32, name="den")
                    pts = [None] * nkt

                    # band tile r has its first 128r q-columns fully masked;
                    # bf16 matmuls run full-rate at any width, so trim exactly
                    def qlo(kt):
                        r = kt - 4 * j
                        return 128 * r if 0 < r < 4 else 0

                    # Band (masked diagonal) tiles first.  Their causal mask
                    # is accumulated into the score psum by an extra identity
                    # matmul (additive -1e5), so exp lands already masked and
                    # the PV matmul chains only through exp.  Denominators:
                    # the 4 band tiles sum on the DVE into one bf16 acc (one
                    # ones-matmul), full tiles sum in bf16 pairs.
                    order = list(range(4 * j, nkt)) + list(range(0, 4 * j))

                    def score(kt):
                        lo = qlo(kt)
                        r = kt - 4 * j
                        band = 0 <= r < 4
                        qk = psQK.tile([P, 512], F32, name="qk")
                        nc.tensor.matmul(qk[:, lo:], ktile(kt),
                                         qs[j][:, h, lo:],
                                         start=True, stop=not band)
                        if band:
                            nc.tensor.matmul(qk[:, lo:lo + 128], ident,
                                             amasks[:, r, lo:lo + 128],
                                             start=False, stop=True,
                                             skip_group_check=True)
                        pt = ppool.tile([P, 512], BF16, name="pt")
                        nc.scalar.activation(pt[:, lo:], qk[:, lo:], EXP,
                                             scale=SCALE)
                        pts[kt] = pt

                    def accum(i):
                        kt = order[i]
                        lo = qlo(kt)
                        nc.tensor.matmul(pv[:, lo:], vtile(kt), pts[kt][:, lo:],
                                         start=(i == 0), stop=(i == nkt - 1))
                        if i == 3:
                            # band group: acc = sum of the 4 masked band tiles
                            acc = bcpool.tile([P, 512], BF16, name="acc")
                            nc.vector.tensor_copy(acc, pts[order[0]])
                            for r in range(1, 4):
                                lor = 128 * r
                                nc.vector.tensor_add(
                                    acc[:, lor:], acc[:, lor:],
                                    pts[order[r]][:, lor:])
                            nc.tensor.matmul(den, ones_bf, acc,
                                             start=True, stop=(nkt == 4))
                        elif i > 3 and (i - 4) % 2 == 1:
                            acc = bcpool.tile([P, 512], BF16, name="acc")
                            nc.vector.tensor_add(acc, pts[order[i - 1]],
                                                 pts[order[i]])
                            nc.tensor.matmul(den, ones_bf, acc,
                                             start=False, stop=(i == nkt - 1))

                    # scores run two tiles ahead of PV so a PV matmul never
                    # blocks the in-order PE queue waiting on its exp
                    score(order[0])
                    score(order[1])
                    for i in range(2, nkt):
                        score(order[i])
                        accum(i - 2)
                        if i % 2 == 0:
                            emit_filler()
                    accum(nkt - 2)
                    accum(nkt - 1)

                    rec_sb = bcpool.tile([P, 512], F32, name="rec_sb")
                    nc.vector.reciprocal_approx_fast(rec_sb, den)
                    nc.vector.tensor_mul(attn_sb[:, h, sl], pv, rec_sb)

                # queue this slice's output projection; slice j+1's heads
                # absorb it as filler between accumulation steps
                for st in range(4 * j, 4 * (j + 1)):
                    for e in range(NS):
                        fillers.append(make_unit(st, e))

            while fillers:
                emit_filler()

    nc.compile()
    return nc


def _consts_array(freqs_cos, freqs_sin):
    c = np.zeros((P, NCONST), NPBF16)
    rt = np.zeros((P, P), np.float32)
    idx = np.arange(0, P, 2)
    rt[idx, idx + 1] = 1.0    # (R.T)[2j, 2j+1] = +1
    rt[idx + 1, idx] = -1.0   # (R.T)[2j+1, 2j] = -1
    c[:, C_RT:C_RT + P] = rt.astype(NPBF16)
    c[:, C_ID:C_ID + P] = np.eye(P, dtype=np.float32).astype(NPBF16)
    c[:, C_ONES:C_ONES + P] = np.float32(1.0).astype(NPBF16)
    c[:, C_COS:C_COS + S] = np.repeat(
        np.asarray(freqs_cos, np.float32).T, 2, axis=0).astype(NPBF16)
    c[:, C_SIN:C_SIN + S] = np.repeat(
        np.asarray(freqs_sin, np.float32).T, 2, axis=0).astype(NPBF16)
    ki = np.arange(P)[:, None]
    qi = np.arange(512)[None, :]
    for r in range(4):
        c[:, C_AM + 512 * r:C_AM + 512 * (r + 1)] = np.where(
            ki <= qi - P * r, 0.0, MASK_NEG).astype(NPBF16)
    return c


def _in_maps(x, wq, wk, wv, wo, freqs_cos, freqs_sin):
    x = np.asarray(x, np.float32)
    wq = np.asarray(wq, np.float32)
    wk = np.asarray(wk, np.float32)
    wv = np.asarray(wv, np.float32)
    wo = np.asarray(wo, np.float32)
    consts = _consts_array(freqs_cos, freqs_sin)
    maps = []
    for core in range(8):
        b, g = divmod(core, 4)
        maps.append({
            "xT": np.ascontiguousarray(x[b].T.astype(NPBF16)),
            "wqT": np.ascontiguousarray(wq[GD * g:GD * (g + 1), :].T.astype(NPBF16)),
            "wkT": np.ascontiguousarray(wk[HD * g:HD * (g + 1), :].T.astype(NPBF16)),
            "wvT": np.ascontiguousarray(wv[HD * g:HD * (g + 1), :].T.astype(NPBF16)),
            "woT": np.ascontiguousarray(wo[:, GD * g:GD * (g + 1)].T.astype(NPBF16)),
            "consts": consts,
        })
    return maps


def _get_nc():
    if "nc" not in _CACHE:
        _CACHE["nc"] = _build()
    return _CACHE["nc"]


def _run(in_maps, trace=False):
    return run_bass_kernel_spmd(_get_nc(), in_maps, core_ids=list(range(8)),
                                trace=trace)


def kernel(x, wq, wk, wv, wo, freqs_cos, freqs_sin):
    res = _run(_in_maps(x, wq, wk, wv, wo, freqs_cos, freqs_sin))
    out = np.zeros((B, S, DIM), np.float32)
    for core in range(8):
        b = core // 4
        out[b] += res.results[core]["out"].astype(np.float32)
    return out


# revision 27
# speedup vs baseline: 1.1682x; 1.1682x over previous
"""GQA causal attention (B=2, S=2048, D=2048, 16 q heads / 4 kv heads, RoPE)
for 8 Trainium2 NeuronCores.

Sharding: core i = (batch b = i//4, kv-head group g = i%4). Each core computes
its group's Q/K/V projections, RoPE, causal attention and the partial output
projection; the host sums the 4 per-group partials per batch.

On-core layout is fully "transposed" (features on partitions):
  xT [D, S], QT/KT [d, S] -> QK scores land as [k, q], softmax runs along k
  (partitions) with the denominator computed by an all-ones matmul, and PV
  accumulates out^T [d, q] directly in PSUM. The final projection contracts
  over the group's 512 head-dims on partitions.
All data is bf16 (fp32r runs in single-pass HIGH mode anyway, so precision is
comparable); matmuls accumulate in fp32 PSUM. bf16 halves HBM traffic and has
no moving>=256 constraint, so causal band tiles are trimmed to exact width.

The causal mask is folded into the score PSUM by an identity-stationary
matmul that accumulates -1e5 at masked positions, so exp output is already
masked (zero there) and the PV matmul never waits on a vector-engine mask.
Softmax denominators batch on the DVE in bf16: the 4 band tiles sum into one
accumulator (one ones-matmul), full tiles sum in pairs.
q/k/v live in per-slice tiles so slice-0 attention never waits on slice-3
RoPE or transposes.
"""

import sys
import types

sys.path.insert(0, "/opt/trn_rl_repo")

# If tracing is ever requested (e.g. BASS_TRACE=1 in the environment),
# concourse needs antenv.axon_hooks, which this image lacks; provide it.
try:
    import antenv  # noqa: F401

    if "antenv.axon_hooks" not in sys.modules:
        from trn_agent_boot.trn_boot import _ntff_profile_via_ctypes

        _mod = types.ModuleType("antenv.axon_hooks")
        _hook = _ntff_profile_via_ctypes("/opt/axon/libaxon_pjrt.so")
        _mod.get_axon_ntff_profile_hook = lambda: _hook
        sys.modules["antenv.axon_hooks"] = _mod
except Exception:
    pass

import ml_dtypes
import numpy as np
from contextlib import ExitStack

import concourse.bacc as bacc
import concourse.mybir as mybir
import concourse.tile as tile
from concourse.bass_utils import run_bass_kernel_spmd

B, S, DIM = 2, 2048, 2048
N_HEADS, N_KV, HD = 16, 4, 128
HPG = N_HEADS // N_KV      # q heads per kv group
GD = HPG * HD              # 512 = group width
P = 128
NS = S // 512              # 4 s-slices of 512
NC = DIM // P              # 16 contraction chunks of 128
NKT = S // P               # 16 k tiles
F32 = mybir.dt.float32
BF16 = mybir.dt.bfloat16
NPBF16 = ml_dtypes.bfloat16
SCALE = 1.0 / float(np.sqrt(HD))
MASK_NEG = -100000.0

# bf16 consts column layout
C_RT = 0          # [128]  RoPE rotation (R.T)
C_ID = 128        # [128]  identity
C_ONES = 256      # [128]  all-ones
C_COS = 384       # [2048] cos, repeated x2 along d
C_SIN = 2432      # [2048]
C_AM = 4480       # [4*512] causal band masks, additive 0/-1e5
NCONST = 6528

_CACHE = {}


def _build():
    nc = bacc.Bacc()
    xT = nc.dram_tensor("xT", [DIM, S], BF16, kind="ExternalInput")
    wqT = nc.dram_tensor("wqT", [DIM, GD], BF16, kind="ExternalInput")
    wkT = nc.dram_tensor("wkT", [DIM, HD], BF16, kind="ExternalInput")
    wvT = nc.dram_tensor("wvT", [DIM, HD], BF16, kind="ExternalInput")
    woT = nc.dram_tensor("woT", [GD, DIM], BF16, kind="ExternalInput")
    consts = nc.dram_tensor("consts", [P, NCONST], BF16, kind="ExternalInput")
    out = nc.dram_tensor("out", [S, DIM], BF16, kind="ExternalOutput")

    EXP = mybir.ActivationFunctionType.Exp

    with tile.TileContext(nc) as tc, ExitStack() as ctx:
        cpool = ctx.enter_context(tc.tile_pool(name="consts", bufs=1))
        persist = ctx.enter_context(tc.tile_pool(name="persist", bufs=1))

        consts_sb = cpool.tile([P, NCONST], BF16, name="consts_sb")
        rt = consts_sb[:, C_RT:C_RT + 128]
        ident = consts_sb[:, C_ID:C_ID + 128]
        ones_bf = consts_sb[:, C_ONES:C_ONES + 128]
        cosf = consts_sb[:, C_COS:C_COS + S]
        sinf = consts_sb[:, C_SIN:C_SIN + S]
        amasks = consts_sb[:, C_AM:C_AM + 4 * 512].rearrange("p (r q) -> p r q", r=4)

        wo_sb = persist.tile([P, HPG, DIM], BF16, name="wo_sb")
        # per-slice q/k/v tiles: slice j's attention only depends on slice j's
        # RoPE/transposes, not on the last phase-1 write to one big tile
        qs = [persist.tile([P, HPG, 512], BF16, name=f"qs{j}") for j in range(NS)]
        ks = [persist.tile([P, 512], BF16, name=f"ks{j}") for j in range(NS)]
        vs = [persist.tile([P, 4, HD], BF16, name=f"vs{j}") for j in range(NS)]
        vt_sb = persist.tile([P, S], BF16, name="vt_sb")

        def ktile(kt):
            return ks[kt // 4][:, P * (kt % 4):P * (kt % 4 + 1)]

        def vtile(kt):
            return vs[kt // 4][:, kt % 4, :]

        # ---- Phase 1: QKV projections + RoPE + V transpose, per s-slice ----
        with ExitStack() as p1:
            wpool = p1.enter_context(tc.tile_pool(name="wqkv", bufs=1))
            xpool = p1.enter_context(tc.tile_pool(name="xs", bufs=3))
            tmpp = p1.enter_context(tc.tile_pool(name="ropetmp", bufs=3))
            psA = p1.enter_context(tc.tile_pool(name="psA", bufs=1, space="PSUM"))
            psRT = p1.enter_context(tc.tile_pool(name="psRT", bufs=1, space="PSUM"))

            wq_sb = wpool.tile([P, NC, GD], BF16, name="wq_sb")
            wk_sb = wpool.tile([P, NC, HD], BF16, name="wk_sb")
            wv_sb = wpool.tile([P, NC, HD], BF16, name="wv_sb")

            warm_sb = wpool.tile([P, 512], BF16, name="warm_sb")
            nc.vector.memset(warm_sb, 0.0)
            for _ in range(3):
                wps = psRT.tile([P, 512], F32, name="warm_ps", tag="rt")
                nc.tensor.matmul(wps, warm_sb[:, :P], warm_sb, start=True, stop=True)

            def dma_consts(lo, n):
                nc.sync.dma_start(out=consts_sb[:, lo:lo + n],
                                  in_=consts[:, lo:lo + n])

            def dma_wq_chunk(cc):
                nc.sync.dma_start(
                    out=wq_sb[:, 4 * cc:4 * (cc + 1), :],
                    in_=wqT[512 * cc:512 * (cc + 1), :]
                    .rearrange("(c p) h -> p c h", p=P))

            def dma_xs(xs, j, cc, engine=None):
                (engine or nc.sync).dma_start(
                    out=xs,
                    in_=xT[512 * cc:512 * (cc + 1), 512 * j:512 * (j + 1)]
                    .rearrange("(c p) s -> p c s", p=P))

            prefetched = None
            for j in range(NS):
                ps = [psA.tile([P, 512], F32, name=f"proj{t}",
                               bufs=2 if t == 0 else 1) for t in range(6)]
                j0_tiles = {}
                for cc in range(4):
                    if cc == 0 and j > 0:
                        xs = prefetched
                    elif j == 0 and cc in j0_tiles:
                        xs = j0_tiles[cc]
                    else:
                        xs = xpool.tile([P, 4, 512], BF16, name="xs")
                    if j == 0 and cc == 0:
                        # interleave the first x columns and q-weight chunks in
                        # small pieces, issued from three engines in parallel
                        # (SP descriptor generation is ~1us per DMA)
                        for c4 in range(4):
                            nc.sync.dma_start(
                                out=xs[:, c4, :],
                                in_=xT[128 * c4:128 * (c4 + 1), 0:512]
                                .rearrange("(c p) s -> p c s", p=P)[:, 0, :])
                            nc.gpsimd.dma_start(
                                out=wq_sb[:, c4, :],
                                in_=wqT[128 * c4:128 * (c4 + 1), :]
                                .rearrange("(c p) h -> p c h", p=P)[:, 0, :])
                        nc.scalar.dma_start(
                            out=wk_sb, in_=wkT[:, :]
                            .rearrange("(c p) h -> p c h", p=P))
                        nc.scalar.dma_start(
                            out=wv_sb, in_=wvT[:, :]
                            .rearrange("(c p) h -> p c h", p=P))
                        nc.gpsimd.dma_start(
                            out=consts_sb[:, C_RT:C_RT + 384],
                            in_=consts[:, C_RT:C_RT + 384])
                        nc.scalar.dma_start(
                            out=consts_sb[:, C_COS:C_COS + 512],
                            in_=consts[:, C_COS:C_COS + 512])
                        nc.gpsimd.dma_start(
                            out=consts_sb[:, C_SIN:C_SIN + 512],
                            in_=consts[:, C_SIN:C_SIN + 512])
                    elif not (j == 0 and cc in j0_tiles) and not (cc == 0 and j > 0):
                        dma_xs(xs, j, cc)
                    if j == 0 and cc + 1 < 4 and cc + 1 not in j0_tiles:
                        nxt = xpool.tile([P, 4, 512], BF16, name="xs")
                        dma_xs(nxt, 0, cc + 1,
                               engine=(None, nc.scalar, nc.gpsimd)[cc])
                        dma_wq_chunk(cc + 1)
                        j0_tiles[cc + 1] = nxt
                    for c4 in range(4):
                        c = 4 * cc + c4
                        first = c == 0
                        last = c == NC - 1
                        for t in range(HPG):
                            nc.tensor.matmul(
                                ps[t], wq_sb[:, c, 128 * t:128 * (t + 1)],
                                xs[:, c4, :], start=first, stop=last)
                        nc.tensor.matmul(ps[4], wk_sb[:, c, :], xs[:, c4, :],
                                         start=first, stop=last)
                        nc.tensor.matmul(ps[5], wv_sb[:, c, :], xs[:, c4, :],
                                         start=first, stop=last)
                if j + 1 < NS:
                    xs_pre = xpool.tile([P, 4, 512], BF16, name="xs")
                    dma_xs(xs_pre, j + 1, 0)
                    prefetched = xs_pre
                else:
                    prefetched = None
                for t in range(HPG):
                    nc.scalar.copy(qs[j][:, t, :], ps[t])
                nc.scalar.copy(ks[j], ps[4])
                nc.scalar.copy(vt_sb[:, 512 * j:512 * (j + 1)], ps[5])
                if j == 0:
                    # attention constants + next slice's cos/sin
                    dma_consts(C_AM, 4 * 512)
                elif j == 1:
                    nc.sync.dma_start(
                        out=wo_sb,
                        in_=woT[:, :].rearrange("(c p) e -> p c e", p=P))
                if j + 1 < NS:
                    dma_consts(C_COS + 512 * (j + 1), 512)
                    dma_consts(C_SIN + 512 * (j + 1), 512)

                # RoPE for this slice (4 q heads + k); the cos-mul runs on
                # gpsimd so the DVE only carries the rot*sin mul and the add
                sl = slice(512 * j, 512 * (j + 1))
                for t in range(HPG + 1):
                    src = qs[j][:, t, :] if t < HPG else ks[j]
                    t2 = tmpp.tile([P, 512], BF16, name="t2")
                    nc.gpsimd.tensor_mul(t2, src, cosf[:, sl])
                    rot = psRT.tile([P, 512], F32, name="rot", tag="rt")
                    nc.tensor.matmul(rot, rt, src, start=True, stop=True)
                    t1 = tmpp.tile([P, 512], F32, name="t1")
                    nc.vector.tensor_mul(t1, rot, sinf[:, sl])
                    nc.vector.tensor_add(src, t1, t2)

                # V transpose (slice 0 only; kt 4..15 run as phase-2
                # filler work during the latency-bound j=0/j=1 heads)
                if j == 0:
                    for kt in range(4):
                        trb = psRT.tile([P, 512], F32, name="trb", tag="rt")
                        tr = trb.bitcast(BF16)[:, :P]
                        nc.tensor.transpose(tr, vt_sb[:, P * kt:P * (kt + 1)],
                                            ident)
                        nc.scalar.copy(vtile(kt), tr)

        # ---- Phase 2: attention (j outer) + interleaved output projection ----
        with ExitStack() as p3:
            ppool = p3.enter_context(tc.tile_pool(name="ptiles", bufs=12))
            bcpool = p3.enter_context(tc.tile_pool(name="bc", bufs=4))
            attnp = p3.enter_context(tc.tile_pool(name="attn", bufs=1))
            outp = p3.enter_context(tc.tile_pool(name="outp", bufs=6))
            psQK = p3.enter_context(tc.tile_pool(name="psQK", bufs=4, space="PSUM"))
            psPV = p3.enter_context(tc.tile_pool(name="psPV", bufs=1, space="PSUM"))
            psDN = p3.enter_context(tc.tile_pool(name="psDN", bufs=1, space="PSUM"))
            psO = p3.enter_context(tc.tile_pool(name="psO", bufs=2, space="PSUM"))

            attn_sb = attnp.tile([P, HPG, S], BF16, name="attn_sb")

            fillers = []

            def make_unit(st, e):
                def unit():
                    ops = psO.tile([P, 512], F32, name="ops")
                    for hc in range(HPG):
                        nc.tensor.matmul(
                            ops, attn_sb[:, hc, P * st:P * (st + 1)],
                            wo_sb[:, hc, 512 * e:512 * (e + 1)],
                            start=(hc == 0), stop=(hc == HPG - 1))
                    osb = outp.tile([P, 512], BF16, name="osb")
                    nc.vector.tensor_copy(osb, ops)
                    eng = nc.sync if (st + e) % 2 == 0 else nc.gpsimd
                    eng.dma_start(
                        out=out[P * st:P * (st + 1), 512 * e:512 * (e + 1)],
                        in_=osb)
                return unit

            tfillers = list(range(4, NKT))

            def emit_transpose():
                kt = tfillers.pop(0)
                trb = psO.tile([P, 512], F32, name="ops")
                tr = trb.bitcast(BF16)[:, :P]
                nc.tensor.transpose(tr, vt_sb[:, P * kt:P * (kt + 1)], ident)
                nc.vector.tensor_copy(vtile(kt), tr)

            def emit_filler():
                if tfillers:
                    emit_transpose()
                    if tfillers:
                        emit_transpose()
                elif fillers:
                    fillers.pop(0)()

            for j in range(NS):
                sl = slice(512 * j, 512 * (j + 1))
                nkt = 4 * (j + 1)
                for h in range(HPG):
                    pv = psPV.tile([P, 512], F32, name="pv")
                    den = psDN.tile([P, 512], F32, name="den")
                    pts = [None] * nkt

                    # band tile r has its first 128r q-columns fully masked;
                    # bf16 matmuls run full-rate at any width, so trim exactly
                    def qlo(kt):
                        r = kt - 4 * j
                        return 128 * r if 0 < r < 4 else 0

                    # Band (masked diagonal) tiles first.  Their causal mask
                    # is accumulated into the score psum by an extra identity
                    # matmul (additive -1e5), so exp lands already masked and
                    # the PV matmul chains only through exp.  Denominators:
                    # the 4 band tiles sum on the DVE into one bf16 acc (one
                    # ones-matmul), full tiles sum in bf16 pairs.
                    order = list(range(4 * j, nkt)) + list(range(0, 4 * j))

                    def score(kt):
                        lo = qlo(kt)
                        r = kt - 4 * j
                        band = 0 <= r < 4
                        qk = psQK.tile([P, 512], F32, name="qk")
                        nc.tensor.matmul(qk[:, lo:], ktile(kt),
                                         qs[j][:, h, lo:],
                                         start=True, stop=not band)
                        if band:
                            nc.tensor.matmul(qk[:, lo:lo + 128], ident,
                                             amasks[:, r, lo:lo + 128],
                                             start=False, stop=True,
                                             skip_group_check=True)
                        pt = ppool.tile([P, 512], BF16, name="pt")
                        nc.scalar.activation(pt[:, lo:], qk[:, lo:], EXP,
                                             scale=SCALE)
                        pts[kt] = pt

                    def accum(i):
                        kt = order[i]
                        lo = qlo(kt)
                        nc.tensor.matmul(pv[:, lo:], vtile(kt), pts[kt][:, lo:],
                                         start=(i == 0), stop=(i == nkt - 1))
                        if i == 3:
                            # band group: acc = sum of the 4 masked band tiles
                            acc = bcpool.tile([P, 512], BF16, name="acc")
                            nc.vector.tensor_copy(acc, pts[order[0]])
                            for r in range(1, 4):
                                lor = 128 * r
                                nc.vector.tensor_add(
                                    acc[:, lor:], acc[:, lor:],
                                    pts[order[r]][:, lor:])
                            nc.tensor.matmul(den, ones_bf, acc,
                                             start=True, stop=(nkt == 4))
                        elif i > 3 and (i - 4) % 2 == 1:
                            acc = bcpool.tile([P, 512], BF16, name="acc")
                            nc.vector.tensor_add(acc, pts[order[i - 1]],
                                                 pts[order[i]])
                            nc.tensor.matmul(den, ones_bf, acc,
                                             start=False, stop=(i == nkt - 1))

                    # scores run two tiles ahead of PV so a PV matmul never
                    # blocks the in-order PE queue waiting on its exp
                    score(order[0])
                    score(order[1])
                    for i in range(2, nkt):
                        score(order[i])
                        accum(i - 2)
                        if i % 2 == 0:
                            emit_filler()
                    accum(nkt - 2)
                    accum(nkt - 1)

                    rec_sb = bcpool.tile([P, 512], F32, name="rec_sb")
                    nc.vector.reciprocal_approx_fast(rec_sb, den)
                    nc.vector.tensor_mul(attn_sb[:, h, sl], pv, rec_sb)

                # queue this slice's output projection; slice j+1's heads
                # absorb it as filler between accumulation steps
                for st in range(4 * j, 4 * (j + 1)):
                    for e in range(NS):
                        fillers.append(make_unit(st, e))

            while fillers:
                emit_filler()

    nc.compile()
    return nc


def _consts_array(freqs_cos, freqs_sin):
    c = np.zeros((P, NCONST), NPBF16)
    rt = np.zeros((P, P), np.float32)
    idx = np.arange(0, P, 2)
    rt[idx, idx + 1] = 1.0    # (R.T)[2j, 2j+1] = +1
    rt[idx + 1, idx] = -1.0   # (R.T)[2j+1, 2j] = -1
    c[:, C_RT:C_RT + P] = rt.astype(NPBF16)
    c[:, C_ID:C_ID + P] = np.eye(P, dtype=np.float32).astype(NPBF16)
    c[:, C_ONES:C_ONES + P] = np.float32(1.0).astype(NPBF16)
    c[:, C_COS:C_COS + S] = np.repeat(
        np.asarray(freqs_cos, np.float32).T, 2, axis=0).astype(NPBF16)
    c[:, C_SIN:C_SIN + S] = np.repeat(
        np.asarray(freqs_sin, np.float32).T, 2, axis=0).astype(NPBF16)
    ki = np.arange(P)[:, None]
    qi = np.arange(512)[None, :]
    for r in range(4):
        c[:, C_AM + 512 * r:C_AM + 512 * (r + 1)] = np.where(
            ki <= qi - P * r, 0.0, MASK_NEG).astype(NPBF16)
    return c


def _in_maps(x, wq, wk, wv, wo, freqs_cos, freqs_sin):
    x = np.asarray(x, np.float32)
    wq = np.asarray(wq, np.float32)
    wk = np.asarray(wk, np.float32)
    wv = np.asarray(wv, np.float32)
    wo = np.asarray(wo, np.float32)
    consts = _consts_array(freqs_cos, freqs_sin)
    maps = []
    for core in range(8):
        b, g = divmod(core, 4)
        maps.append({
            "xT": np.ascontiguousarray(x[b].T.astype(NPBF16)),
            "wqT": np.ascontiguousarray(wq[GD * g:GD * (g + 1), :].T.astype(NPBF16)),
            "wkT": np.ascontiguousarray(wk[HD * g:HD * (g + 1), :].T.astype(NPBF16)),
            "wvT": np.ascontiguousarray(wv[HD * g:HD * (g + 1), :].T.astype(NPBF16)),
            "woT": np.ascontiguousarray(wo[:, GD * g:GD * (g + 1)].T.astype(NPBF16)),
            "consts": consts,
        })
    return maps


def _get_nc():
    if "nc" not in _CACHE:
        _CACHE["nc"] = _build()
    return _CACHE["nc"]


def _run(in_maps, trace=False):
    return run_bass_kernel_spmd(_get_nc(), in_maps, core_ids=list(range(8)),
                                trace=trace)


def kernel(x, wq, wk, wv, wo, freqs_cos, freqs_sin):
    res = _run(_in_maps(x, wq, wk, wv, wo, freqs_cos, freqs_sin))
    out = np.zeros((B, S, DIM), np.float32)
    for core in range(8):
        b = core // 4
        out[b] += res.results[core]["out"].astype(np.float32)
    return out


# revision 30
# speedup vs baseline: 1.1732x; 1.0042x over previous
"""GQA causal attention (B=2, S=2048, D=2048, 16 q heads / 4 kv heads, RoPE)
for 8 Trainium2 NeuronCores.

Sharding: core i = (batch b = i//4, kv-head group g = i%4). Each core computes
its group's Q/K/V projections, RoPE, causal attention and the partial output
projection; the host sums the 4 per-group partials per batch.

On-core layout is fully "transposed" (features on partitions):
  xT [D, S], QT/KT [d, S] -> QK scores land as [k, q], softmax runs along k
  (partitions) with the denominator computed by an all-ones matmul, and PV
  accumulates out^T [d, q] directly in PSUM. The final projection contracts
  over the group's 512 head-dims on partitions.
All data is bf16 (fp32r runs in single-pass HIGH mode anyway, so precision is
comparable); matmuls accumulate in fp32 PSUM. bf16 halves HBM traffic and has
no moving>=256 constraint, so causal band tiles are trimmed to exact width.

The causal mask is folded into the score PSUM by an identity-stationary
matmul that accumulates -1e5 at masked positions, so exp output is already
masked (zero there) and the PV matmul never waits on a vector-engine mask.
Softmax denominators batch on the DVE in bf16: the 4 band tiles sum into one
accumulator (one ones-matmul), full tiles sum in pairs.
q/k/v live in per-slice tiles so slice-0 attention never waits on slice-3
RoPE or transposes.
"""

import sys
import types

sys.path.insert(0, "/opt/trn_rl_repo")

# If tracing is ever requested (e.g. BASS_TRACE=1 in the environment),
# concourse needs antenv.axon_hooks, which this image lacks; provide it.
try:
    import antenv  # noqa: F401

    if "antenv.axon_hooks" not in sys.modules:
        from trn_agent_boot.trn_boot import _ntff_profile_via_ctypes

        _mod = types.ModuleType("antenv.axon_hooks")
        _hook = _ntff_profile_via_ctypes("/opt/axon/libaxon_pjrt.so")
        _mod.get_axon_ntff_profile_hook = lambda: _hook
        sys.modules["antenv.axon_hooks"] = _mod
except Exception:
    pass

import ml_dtypes
import numpy as np
from contextlib import ExitStack

import concourse.bacc as bacc
import concourse.mybir as mybir
import concourse.tile as tile
from concourse.bass_utils import run_bass_kernel_spmd

B, S, DIM = 2, 2048, 2048
N_HEADS, N_KV, HD = 16, 4, 128
HPG = N_HEADS // N_KV      # q heads per kv group
GD = HPG * HD              # 512 = group width
P = 128
NS = S // 512              # 4 s-slices of 512
NC = DIM // P              # 16 contraction chunks of 128
NKT = S // P               # 16 k tiles
F32 = mybir.dt.float32
BF16 = mybir.dt.bfloat16
NPBF16 = ml_dtypes.bfloat16
SCALE = 1.0 / float(np.sqrt(HD))
MASK_NEG = -100000.0

# bf16 consts column layout
C_RT = 0          # [128]  RoPE rotation (R.T)
C_ID = 128        # [128]  identity
C_ONES = 256      # [128]  all-ones
C_COS = 384       # [2048] cos, repeated x2 along d
C_SIN = 2432      # [2048]
C_AM = 4480       # [4*512] causal band masks, additive 0/-1e5
NCONST = 6528

_CACHE = {}


def _build():
    nc = bacc.Bacc()
    xT = nc.dram_tensor("xT", [DIM, S], BF16, kind="ExternalInput")
    wqT = nc.dram_tensor("wqT", [DIM, GD], BF16, kind="ExternalInput")
    wkT = nc.dram_tensor("wkT", [DIM, HD], BF16, kind="ExternalInput")
    wvT = nc.dram_tensor("wvT", [DIM, HD], BF16, kind="ExternalInput")
    woT = nc.dram_tensor("woT", [GD, DIM], BF16, kind="ExternalInput")
    consts = nc.dram_tensor("consts", [P, NCONST], BF16, kind="ExternalInput")
    out = nc.dram_tensor("out", [S, DIM], BF16, kind="ExternalOutput")

    EXP = mybir.ActivationFunctionType.Exp

    with tile.TileContext(nc) as tc, ExitStack() as ctx:
        cpool = ctx.enter_context(tc.tile_pool(name="consts", bufs=1))
        persist = ctx.enter_context(tc.tile_pool(name="persist", bufs=1))

        consts_sb = cpool.tile([P, NCONST], BF16, name="consts_sb")
        rt = consts_sb[:, C_RT:C_RT + 128]
        ident = consts_sb[:, C_ID:C_ID + 128]
        ones_bf = consts_sb[:, C_ONES:C_ONES + 128]
        cosf = consts_sb[:, C_COS:C_COS + S]
        sinf = consts_sb[:, C_SIN:C_SIN + S]
        amasks = consts_sb[:, C_AM:C_AM + 4 * 512].rearrange("p (r q) -> p r q", r=4)

        wo_sb = persist.tile([P, HPG, DIM], BF16, name="wo_sb")
        # per-slice q/k/v tiles: slice j's attention only depends on slice j's
        # RoPE/transposes, not on the last phase-1 write to one big tile
        qs = [persist.tile([P, HPG, 512], BF16, name=f"qs{j}") for j in range(NS)]
        ks = [persist.tile([P, 512], BF16, name=f"ks{j}") for j in range(NS)]
        vs = [persist.tile([P, 4, HD], BF16, name=f"vs{j}") for j in range(NS)]
        vt_sb = persist.tile([P, S], BF16, name="vt_sb")

        def ktile(kt):
            return ks[kt // 4][:, P * (kt % 4):P * (kt % 4 + 1)]

        def vtile(kt):
            return vs[kt // 4][:, kt % 4, :]

        # ---- Phase 1: QKV projections + RoPE + V transpose, per s-slice ----
        with ExitStack() as p1:
            wpool = p1.enter_context(tc.tile_pool(name="wqkv", bufs=1))
            xpool = p1.enter_context(tc.tile_pool(name="xs", bufs=3))
            tmpp = p1.enter_context(tc.tile_pool(name="ropetmp", bufs=3))
            psA = p1.enter_context(tc.tile_pool(name="psA", bufs=1, space="PSUM"))
            psRT = p1.enter_context(tc.tile_pool(name="psRT", bufs=1, space="PSUM"))

            wq_sb = wpool.tile([P, NC, GD], BF16, name="wq_sb")
            wk_sb = wpool.tile([P, NC, HD], BF16, name="wk_sb")
            wv_sb = wpool.tile([P, NC, HD], BF16, name="wv_sb")

            warm_sb = wpool.tile([P, 512], BF16, name="warm_sb")
            nc.vector.memset(warm_sb, 0.0)
            for _ in range(3):
                wps = psRT.tile([P, 512], F32, name="warm_ps", tag="rt")
                nc.tensor.matmul(wps, warm_sb[:, :P], warm_sb, start=True, stop=True)

            def dma_consts(lo, n):
                nc.sync.dma_start(out=consts_sb[:, lo:lo + n],
                                  in_=consts[:, lo:lo + n])

            def dma_wq_chunk(cc):
                nc.sync.dma_start(
                    out=wq_sb[:, 4 * cc:4 * (cc + 1), :],
                    in_=wqT[512 * cc:512 * (cc + 1), :]
                    .rearrange("(c p) h -> p c h", p=P))

            def dma_xs(xs, j, cc, engine=None):
                (engine or nc.sync).dma_start(
                    out=xs,
                    in_=xT[512 * cc:512 * (cc + 1), 512 * j:512 * (j + 1)]
                    .rearrange("(c p) s -> p c s", p=P))

            prefetched = None
            for j in range(NS):
                ps = [psA.tile([P, 512], F32, name=f"proj{t}",
                               bufs=2 if t == 0 else 1) for t in range(6)]
                j0_tiles = {}
                for cc in range(4):
                    if cc == 0 and j > 0:
                        xs = prefetched
                    elif j == 0 and cc in j0_tiles:
                        xs = j0_tiles[cc]
                    else:
                        xs = xpool.tile([P, 4, 512], BF16, name="xs")
                    if j == 0 and cc == 0:
                        # interleave the first x columns and q-weight chunks in
                        # small pieces, issued from three engines in parallel
                        # (SP descriptor generation is ~1us per DMA)
                        for c4 in range(4):
                            nc.sync.dma_start(
                                out=xs[:, c4, :],
                                in_=xT[128 * c4:128 * (c4 + 1), 0:512]
                                .rearrange("(c p) s -> p c s", p=P)[:, 0, :])
                            nc.gpsimd.dma_start(
                                out=wq_sb[:, c4, :],
                                in_=wqT[128 * c4:128 * (c4 + 1), :]
                                .rearrange("(c p) h -> p c h", p=P)[:, 0, :])
                        nc.scalar.dma_start(
                            out=wk_sb, in_=wkT[:, :]
                            .rearrange("(c p) h -> p c h", p=P))
                        nc.scalar.dma_start(
                            out=wv_sb, in_=wvT[:, :]
                            .rearrange("(c p) h -> p c h", p=P))
                        nc.gpsimd.dma_start(
                            out=consts_sb[:, C_RT:C_RT + 384],
                            in_=consts[:, C_RT:C_RT + 384])
                        nc.scalar.dma_start(
                            out=consts_sb[:, C_COS:C_COS + 512],
                            in_=consts[:, C_COS:C_COS + 512])
                        nc.gpsimd.dma_start(
                            out=consts_sb[:, C_SIN:C_SIN + 512],
                            in_=consts[:, C_SIN:C_SIN + 512])
                    elif not (j == 0 and cc in j0_tiles) and not (cc == 0 and j > 0):
                        dma_xs(xs, j, cc)
                    if j == 0 and cc + 1 < 4 and cc + 1 not in j0_tiles:
                        nxt = xpool.tile([P, 4, 512], BF16, name="xs")
                        dma_xs(nxt, 0, cc + 1,
                               engine=(None, nc.scalar, nc.gpsimd)[cc])
                        dma_wq_chunk(cc + 1)
                        j0_tiles[cc + 1] = nxt
                    for c4 in range(4):
                        c = 4 * cc + c4
                        first = c == 0
                        last = c == NC - 1
                        for t in range(HPG):
                            nc.tensor.matmul(
                                ps[t], wq_sb[:, c, 128 * t:128 * (t + 1)],
                                xs[:, c4, :], start=first, stop=last)
                        nc.tensor.matmul(ps[4], wk_sb[:, c, :], xs[:, c4, :],
                                         start=first, stop=last)
                        nc.tensor.matmul(ps[5], wv_sb[:, c, :], xs[:, c4, :],
                                         start=first, stop=last)
                if j + 1 < NS:
                    xs_pre = xpool.tile([P, 4, 512], BF16, name="xs")
                    dma_xs(xs_pre, j + 1, 0)
                    prefetched = xs_pre
                else:
                    prefetched = None
                for t in range(HPG):
                    nc.scalar.copy(qs[j][:, t, :], ps[t])
                nc.scalar.copy(ks[j], ps[4])
                nc.scalar.copy(vt_sb[:, 512 * j:512 * (j + 1)], ps[5])
                if j == 0:
                    # attention constants + next slice's cos/sin
                    dma_consts(C_AM, 4 * 512)
                elif j == 1:
                    nc.sync.dma_start(
                        out=wo_sb,
                        in_=woT[:, :].rearrange("(c p) e -> p c e", p=P))
                if j + 1 < NS:
                    dma_consts(C_COS + 512 * (j + 1), 512)
                    dma_consts(C_SIN + 512 * (j + 1), 512)

                # RoPE for this slice (4 q heads + k); the cos-mul runs on
                # gpsimd so the DVE only carries the rot*sin mul and the add
                sl = slice(512 * j, 512 * (j + 1))
                for t in range(HPG + 1):
                    src = qs[j][:, t, :] if t < HPG else ks[j]
                    t2 = tmpp.tile([P, 512], BF16, name="t2")
                    nc.gpsimd.tensor_mul(t2, src, cosf[:, sl])
                    rot = psRT.tile([P, 512], F32, name="rot", tag="rt")
                    nc.tensor.matmul(rot, rt, src, start=True, stop=True)
                    t1 = tmpp.tile([P, 512], F32, name="t1")
                    nc.vector.tensor_mul(t1, rot, sinf[:, sl])
                    nc.vector.tensor_add(src, t1, t2)

                # V transpose (slice 0 only; kt 4..15 run as phase-2
                # filler work during the latency-bound j=0/j=1 heads)
                if j == 0:
                    for kt in range(4):
                        trb = psRT.tile([P, 512], F32, name="trb", tag="rt")
                        tr = trb.bitcast(BF16)[:, :P]
                        nc.tensor.transpose(tr, vt_sb[:, P * kt:P * (kt + 1)],
                                            ident)
                        nc.scalar.copy(vtile(kt), tr)

        # ---- Phase 2: attention (j outer) + interleaved output projection ----
        with ExitStack() as p3:
            ppool = p3.enter_context(tc.tile_pool(name="ptiles", bufs=12))
            bcpool = p3.enter_context(tc.tile_pool(name="bc", bufs=4))
            attnp = p3.enter_context(tc.tile_pool(name="attn", bufs=1))
            outp = p3.enter_context(tc.tile_pool(name="outp", bufs=6))
            psQK = p3.enter_context(tc.tile_pool(name="psQK", bufs=4, space="PSUM"))
            psPV = p3.enter_context(tc.tile_pool(name="psPV", bufs=1, space="PSUM"))
            psDN = p3.enter_context(tc.tile_pool(name="psDN", bufs=1, space="PSUM"))
            psO = p3.enter_context(tc.tile_pool(name="psO", bufs=2, space="PSUM"))

            attn_sb = attnp.tile([P, HPG, S], BF16, name="attn_sb")

            fillers = []

            def make_unit(st, e):
                def unit():
                    ops = psO.tile([P, 512], F32, name="ops")
                    for hc in range(HPG):
                        nc.tensor.matmul(
                            ops, attn_sb[:, hc, P * st:P * (st + 1)],
                            wo_sb[:, hc, 512 * e:512 * (e + 1)],
                            start=(hc == 0), stop=(hc == HPG - 1))
                    osb = outp.tile([P, 512], BF16, name="osb")
                    nc.vector.tensor_copy(osb, ops)
                    eng = nc.sync if (st + e) % 2 == 0 else nc.gpsimd
                    eng.dma_start(
                        out=out[P * st:P * (st + 1), 512 * e:512 * (e + 1)],
                        in_=osb)
                return unit

            tfillers = list(range(4, NKT))

            def emit_transpose():
                kt = tfillers.pop(0)
                trb = psO.tile([P, 512], F32, name="ops")
                tr = trb.bitcast(BF16)[:, :P]
                nc.tensor.transpose(tr, vt_sb[:, P * kt:P * (kt + 1)], ident)
                nc.vector.tensor_copy(vtile(kt), tr)

            def emit_filler():
                if tfillers:
                    emit_transpose()
                    if tfillers:
                        emit_transpose()
                elif fillers:
                    fillers.pop(0)()

            for j in range(NS):
                sl = slice(512 * j, 512 * (j + 1))
                nkt = 4 * (j + 1)
                for h in range(HPG):
                    pv = psPV.tile([P, 512], F32, name="pv")
                    den = psDN.tile([P, 512], F32, name="den")
                    pts = [None] * nkt

                    # band tile r has its first 128r q-columns fully masked;
                    # bf16 matmuls run full-rate at any width, so trim exactly
                    def qlo(kt):
                        r = kt - 4 * j
                        return 128 * r if 0 < r < 4 else 0

                    # Band (masked diagonal) tiles first.  Their causal mask
                    # is accumulated into the score psum by an extra identity
                    # matmul (additive -1e5), so exp lands already masked and
                    # the PV matmul chains only through exp.  Denominators:
                    # the 4 band tiles sum on the DVE into one bf16 acc (one
                    # ones-matmul), full tiles sum in bf16 pairs.
                    order = list(range(4 * j, nkt)) + list(range(0, 4 * j))

                    def score(kt):
                        lo = qlo(kt)
                        r = kt - 4 * j
                        band = 0 <= r < 4
                        qk = psQK.tile([P, 512], F32, name="qk")
                        nc.tensor.matmul(qk[:, lo:], ktile(kt),
                                         qs[j][:, h, lo:],
                                         start=True, stop=not band)
                        if band:
                            nc.tensor.matmul(qk[:, lo:lo + 128], ident,
                                             amasks[:, r, lo:lo + 128],
                                             start=False, stop=True,
                                             skip_group_check=True)
                        pt = ppool.tile([P, 512], BF16, name="pt")
                        nc.scalar.activation(pt[:, lo:], qk[:, lo:], EXP,
                                             scale=SCALE)
                        pts[kt] = pt

                    def accum(i):
                        kt = order[i]
                        lo = qlo(kt)
                        nc.tensor.matmul(pv[:, lo:], vtile(kt), pts[kt][:, lo:],
                                         start=(i == 0), stop=(i == nkt - 1))
                        if i == 3:
                            # band group: acc = sum of the 4 masked band tiles
                            acc = bcpool.tile([P, 512], BF16, name="acc")
                            nc.vector.tensor_copy(acc, pts[order[0]])
                            for r in range(1, 4):
                                lor = 128 * r
                                nc.vector.tensor_add(
                                    acc[:, lor:], acc[:, lor:],
                                    pts[order[r]][:, lor:])
                            nc.tensor.matmul(den, ones_bf, acc,
                                             start=True, stop=(nkt == 4))
                        elif i > 3 and (i - 4) % 2 == 1:
                            acc = bcpool.tile([P, 512], BF16, name="acc")
                            nc.vector.tensor_add(acc, pts[order[i - 1]],
                                                 pts[order[i]])
                            nc.tensor.matmul(den, ones_bf, acc,
                                             start=False, stop=(i == nkt - 1))

                    # scores run three tiles ahead of PV (psQK holds 4) so
                    # a PV matmul never blocks the in-order PE queue on exp.
                    # Fillers emit on odd i: j=0's loop is i=3 only, and the
                    # slice-1 transposes MUST land there, before j=1's first
                    # PV issues (its vs read has no later-write dependency)
                    score(order[0])
                    score(order[1])
                    score(order[2])
                    for i in range(3, nkt):
                        score(order[i])
                        accum(i - 3)
                        if i % 2 == 1:
                            emit_filler()
                    accum(nkt - 3)
                    accum(nkt - 2)
                    accum(nkt - 1)

                    rec_sb = bcpool.tile([P, 512], F32, name="rec_sb")
                    nc.vector.reciprocal_approx_fast(rec_sb, den)
                    nc.vector.tensor_mul(attn_sb[:, h, sl], pv, rec_sb)

                # queue this slice's output projection; slice j+1's heads
                # absorb it as filler between accumulation steps
                for st in range(4 * j, 4 * (j + 1)):
                    for e in range(NS):
                        fillers.append(make_unit(st, e))

            while fillers:
                emit_filler()

    nc.compile()
    return nc


def _consts_array(freqs_cos, freqs_sin):
    c = np.zeros((P, NCONST), NPBF16)
    rt = np.zeros((P, P), np.float32)
    idx = np.arange(0, P, 2)
    rt[idx, idx + 1] = 1.0    # (R.T)[2j, 2j+1] = +1
    rt[idx + 1, idx] = -1.0   # (R.T)[2j+1, 2j] = -1
    c[:, C_RT:C_RT + P] = rt.astype(NPBF16)
    c[:, C_ID:C_ID + P] = np.eye(P, dtype=np.float32).astype(NPBF16)
    c[:, C_ONES:C_ONES + P] = np.float32(1.0).astype(NPBF16)
    c[:, C_COS:C_COS + S] = np.repeat(
        np.asarray(freqs_cos, np.float32).T, 2, axis=0).astype(NPBF16)
    c[:, C_SIN:C_SIN + S] = np.repeat(
        np.asarray(freqs_sin, np.float32).T, 2, axis=0).astype(NPBF16)
    ki = np.arange(P)[:, None]
    qi = np.arange(512)[None, :]
    for r in range(4):
        c[:, C_AM + 512 * r:C_AM + 512 * (r + 1)] = np.where(
            ki <= qi - P * r, 0.0, MASK_NEG).astype(NPBF16)
    return c


def _in_maps(x, wq, wk, wv, wo, freqs_cos, freqs_sin):
    x = np.asarray(x, np.float32)
    wq = np.asarray(wq, np.float32)
    wk = np.asarray(wk, np.float32)
    wv = np.asarray(wv, np.float32)
    wo = np.asarray(wo, np.float32)
    consts = _consts_array(freqs_cos, freqs_sin)
    maps = []
    for core in range(8):
        b, g = divmod(core, 4)
        maps.append({
            "xT": np.ascontiguousarray(x[b].T.astype(NPBF16)),
            "wqT": np.ascontiguousarray(wq[GD * g:GD * (g + 1), :].T.astype(NPBF16)),
            "wkT": np.ascontiguousarray(wk[HD * g:HD * (g + 1), :].T.astype(NPBF16)),
            "wvT": np.ascontiguousarray(wv[HD * g:HD * (g + 1), :].T.astype(NPBF16)),
            "woT": np.ascontiguousarray(wo[:, GD * g:GD * (g + 1)].T.astype(NPBF16)),
            "consts": consts,
        })
    return maps


def _get_nc():
    if "nc" not in _CACHE:
        _CACHE["nc"] = _build()
    return _CACHE["nc"]


def _run(in_maps, trace=False):
    return run_bass_kernel_spmd(_get_nc(), in_maps, core_ids=list(range(8)),
                                trace=trace)


def kernel(x, wq, wk, wv, wo, freqs_cos, freqs_sin):
    res = _run(_in_maps(x, wq, wk, wv, wo, freqs_cos, freqs_sin))
    out = np.zeros((B, S, DIM), np.float32)
    for core in range(8):
        b = core // 4
        out[b] += res.results[core]["out"].astype(np.float32)
    return out
